# revision 32
# baseline (speedup 1.0000x reference)
"""BrainRNN forward pass on 8 TRN2 NeuronCores (Bass/Tile, SPMD).

Strategy (tensor-parallel over output neurons, low-precision streaming):
  - Each block's 1024 output neurons are row-sharded 128/core; masks are
    folded into weights on the host.  W_rec/W_skip stream as fp8e4m3,
    everything else as bf16 (absmax err ~7e-3 vs 1.7e-2 budget): cuts
    HBM traffic 51.8MB -> ~14MB per core.
  - Matmuls keep activations as the 64-wide stationary operand and the
    streamed weights as the MOVING operand (1 cyc/row sub-fp32 dtypes).
    Chain-independent blocks share one wide matmul via host-side
    M-interleaved packing (rec23 M=256, rec456 M=384, skip quads M=512),
    amortizing LDWEIGHTS + instruction overhead.
  - Skip contributions are regrouped by SOURCE block c (available after
    AllGather_c) and split into an "urgent" solo targeting the next
    sigmoid and a deferred wide group for later AG windows.
  - Chain per block: sigmoid (PSUM->SBUF bf16), scalar bounce of the
    untransposed [64, 128] shard to DRAM, AllGather (bf16, Shared
    output), one HWDGE X-bar transpose-DMA landing directly in curT
    layout (no PE transpose / DVE copy / rearrange unpack).
  - Output block: each core contracts its own 128 rows of cur7 against
    W_out[:, rows].T (N=512); ReduceScatter yields an 8-row batch shard.
"""

import numpy as np
import ml_dtypes

N = 8192
W = 1024
L = 8
B = 64
IN = 512
OUT = 512
NCORES = 8
RP = W // NCORES        # 128 rows per core per block

BF16 = ml_dtypes.bfloat16
FP8 = ml_dtypes.float8_e4m3

# skip groups: (source c, target blocks) -> one packed dram tensor each.
# "urgent" solos feed the next sigmoid; wide groups fill later AG windows.
SKIP_GROUPS = [
    (0, (2, 3)),
    (0, (4, 5, 6, 7)),
    (1, (2,)),
    (1, (3,)),
    (1, (4, 5, 6, 7)),
    (2, (3,)),
    (2, (4, 5, 6, 7)),
    (3, (4,)),
    (3, (5, 6, 7)),
    (4, (5,)),
    (4, (6, 7)),
    (5, (6,)),
    (5, (7,)),
    (6, (7,)),
]

_BUILT = None


def _pack_group(mats, out_dtype):
    """mats: list of [M_i, K] (same K).  Returns [128, (K/128)*sum(M_i)]
    where chunk k holds [A_0[:,k128].T | A_1[:,k128].T | ...]."""
    K = mats[0].shape[1]
    nk = K // 128
    chunks = []
    for k in range(nk):
        cols = [np.asarray(A[:, k * 128:(k + 1) * 128].T, np.float32)
                for A in mats]
        chunks.append(np.concatenate(cols, axis=1))
    return np.ascontiguousarray(
        np.concatenate(chunks, axis=1).astype(out_dtype))


def _pack(A, out_dtype):
    return _pack_group([A], out_dtype)


def _build():
    import concourse.bass as bass
    import concourse.bacc as bacc
    import concourse.mybir as mybir
    import concourse.tile as tile

    fp32 = mybir.dt.float32
    bf16 = mybir.dt.bfloat16
    fp8 = mybir.dt.float8e4
    AF = mybir.ActivationFunctionType

    nc = bacc.Bacc(
        "TRN2",
        target_bir_lowering=False,
        debug=False,
        enable_asserts=False,
        num_devices=NCORES,
    )

    t_hT = nc.dram_tensor("hT", [128, 64 * B], fp8, kind="ExternalInput")
    t_xT = nc.dram_tensor("xT", [128, 4 * B], bf16, kind="ExternalInput")
    t_win = nc.dram_tensor("win", [128, 4 * RP], bf16, kind="ExternalInput")
    t_rec0 = nc.dram_tensor("rec0", [128, 64 * 128], fp8, kind="ExternalInput")
    t_rec1 = nc.dram_tensor("rec1", [128, 64 * 128], fp8, kind="ExternalInput")
    t_rec23 = nc.dram_tensor("rec23", [128, 64 * 256], fp8, kind="ExternalInput")
    t_rec456 = nc.dram_tensor("rec456", [128, 64 * 384], fp8, kind="ExternalInput")
    t_hid = {
        j: nc.dram_tensor(f"hid{j}", [128, 8 * 128], bf16, kind="ExternalInput")
        for j in range(1, 8)
    }
    t_skip = {
        (c, T): nc.dram_tensor(
            f"skip{c}_{''.join(map(str, T))}", [128, 8 * 128 * len(T)], fp8,
            kind="ExternalInput")
        for c, T in SKIP_GROUPS
    }
    t_biasrow = nc.dram_tensor("biasrow", [1, 8 * 128], bf16, kind="ExternalInput")
    t_wout = nc.dram_tensor("wout", [128, 8 * 512], bf16, kind="ExternalInput")
    t_boutrow = nc.dram_tensor("boutrow", [1, 512], bf16, kind="ExternalInput")
    t_ones = nc.dram_tensor("ones", [1, B], bf16, kind="ExternalInput")
    t_ident = nc.dram_tensor("ident", [B, B], bf16, kind="ExternalInput")
    t_out = nc.dram_tensor("out", [64, 512], fp32, kind="ExternalOutput")

    rg = [list(range(NCORES))]

    with tile.TileContext(nc) as tc:
        with (
            tc.tile_pool(name="const", bufs=1) as constp,
            tc.tile_pool(name="w", bufs=1) as wp,
            tc.tile_pool(name="curs", bufs=1) as curp,
            tc.tile_pool(name="psum", bufs=1, space="PSUM") as psump,
            tc.tile_pool(name="dram", bufs=1, space="DRAM") as dramp,
        ):
            # ---- persistent inputs (DMA emission order = priority) -----
            xT_sb = constp.tile([128, 4 * B], bf16, name="xT_sb", tag="xT")
            nc.sync.dma_start(out=xT_sb, in_=t_xT[:, :])
            win_sb = constp.tile([128, 4 * RP], bf16, name="win_sb", tag="win")
            nc.sync.dma_start(out=win_sb, in_=t_win[:, :])
            biasrow_sb = constp.tile([1, 8 * 128], bf16, name="biasrow_sb", tag="br")
            nc.sync.dma_start(out=biasrow_sb, in_=t_biasrow[:, :])
            ones_sb = constp.tile([1, B], bf16, name="ones_sb", tag="ones")
            nc.sync.dma_start(out=ones_sb, in_=t_ones[:, :])
            ident_sb = constp.tile([B, B], bf16, name="ident_sb", tag="ident")
            nc.sync.dma_start(out=ident_sb, in_=t_ident[:, :])
            hT_sb = constp.tile([128, 64 * B], fp8, name="hT_sb", tag="hT")
            for i in range(2):
                nc.sync.dma_start(
                    out=hT_sb[:, i * 2048:(i + 1) * 2048],
                    in_=t_hT[:, i * 2048:(i + 1) * 2048],
                )

            def stream(dst, src, piece_cols):
                total = src.shape[1]
                o = 0
                while o < total:
                    e = min(o + piece_cols, total)
                    nc.sync.dma_start(out=dst[:, o:e], in_=src[:, o:e])
                    o = e

            # weight tiles (persistent, distinct tags)
            rec0_sb = wp.tile([128, 64 * 128], fp8, name="rec0_sb", tag="rec0")
            stream(rec0_sb, t_rec0, 1, 2048)          # 4 x 256KB
            rec1_sb = wp.tile([128, 64 * 128], fp8, name="rec1_sb", tag="rec1")
            stream(rec1_sb, t_rec1, 1, 4096)
            hid_sb = {}
            hid_sb[1] = wp.tile([128, 8 * 128], bf16, name="hid1_sb", tag="hid1")
            nc.sync.dma_start(out=hid_sb[1], in_=t_hid[1][:, :])
            rec23_sb = wp.tile([128, 64 * 256], fp8, name="rec23_sb", tag="rec23")
            stream(rec23_sb, t_rec23, 1, 4096)
            hid_sb[2] = wp.tile([128, 8 * 128], bf16, name="hid2_sb", tag="hid2")
            nc.sync.dma_start(out=hid_sb[2], in_=t_hid[2][:, :])
            skip_sb = {}

            def skip_load(c, T):
                st = wp.tile(
                    [128, 8 * 128 * len(T)], fp8,
                    name=f"skip{c}_{''.join(map(str, T))}_sb",
                    tag=f"sk{c}_{T[0]}",
                )
                stream(st, t_skip[(c, T)], 1, 4096)
                skip_sb[(c, T)] = st

            skip_load(0, (2, 3))
            rec456_sb = wp.tile([128, 64 * 384], fp8, name="rec456_sb", tag="rec456")
            stream(rec456_sb, t_rec456, 1, 4096)
            skip_load(1, (2,))
            hid_sb[3] = wp.tile([128, 8 * 128], bf16, name="hid3_sb", tag="hid3")
            nc.sync.dma_start(out=hid_sb[3], in_=t_hid[3][:, :])
            skip_load(2, (3,))
            skip_load(1, (3,))
            skip_load(0, (4, 5, 6, 7))
            skip_load(1, (4, 5, 6, 7))
            skip_load(2, (4, 5, 6, 7))
            hid_sb[4] = wp.tile([128, 8 * 128], bf16, name="hid4_sb", tag="hid4")
            nc.sync.dma_start(out=hid_sb[4], in_=t_hid[4][:, :])
            skip_load(3, (4,))
            skip_load(3, (5, 6, 7))
            hid_sb[5] = wp.tile([128, 8 * 128], bf16, name="hid5_sb", tag="hid5")
            nc.sync.dma_start(out=hid_sb[5], in_=t_hid[5][:, :])
            skip_load(4, (5,))
            skip_load(4, (6, 7))
            hid_sb[6] = wp.tile([128, 8 * 128], bf16, name="hid6_sb", tag="hid6")
            nc.sync.dma_start(out=hid_sb[6], in_=t_hid[6][:, :])
            skip_load(5, (6,))
            skip_load(5, (7,))
            hid_sb[7] = wp.tile([128, 8 * 128], bf16, name="hid7_sb", tag="hid7")
            nc.sync.dma_start(out=hid_sb[7], in_=t_hid[7][:, :])
            skip_load(6, (7,))
            wout_sb = constp.tile([128, 8 * 512], bf16, name="wout_sb", tag="wout")
            stream(wout_sb, t_wout, 2048)
            boutrow_sb = constp.tile([1, 512], bf16, name="boutrow_sb", tag="bo")
            nc.sync.dma_start(out=boutrow_sb, in_=t_boutrow[:, :])

            # ---- PSUM accumulators ------------------------------------
            psA = psump.tile([64, 512], fp32, name="psA", tag="psA")  # blocks 0-3
            psB = psump.tile([64, 512], fp32, name="psB", tag="psB")  # blocks 4-7

            def pscol(j):
                ps = psA if j < 4 else psB
                o = (j % 4) * 128
                return ps, o

            # open both accumulators with the bias rows
            nc.tensor.matmul(psA, lhsT=ones_sb[:, :], rhs=biasrow_sb[:, 0:512],
                             start=True, stop=False)
            nc.tensor.matmul(psB, lhsT=ones_sb[:, :], rhs=biasrow_sb[:, 512:1024],
                             start=True, stop=False)

            def mm(ps, col, wid, lhsT, rhs, stop=False):
                nc.tensor.matmul(ps[:, col:col + wid], lhsT=lhsT,
                                 rhs=rhs, start=False, stop=stop)

            # x @ W_in.T into block 0
            for kk in range(4):
                mm(psA, 0, 128, xT_sb[:, kk * B:(kk + 1) * B],
                   win_sb[:, kk * RP:(kk + 1) * RP])

            def rec_mms(tile_sb, col, wid, stop_last=False):
                for kg in range(64):
                    mm(psA if col < 512 else psB, col % 512, wid,
                       hT_sb[:, kg * B:(kg + 1) * B],
                       tile_sb[:, kg * wid:(kg + 1) * wid],
                       stop=stop_last and kg == 63)

            curT = [None] * 8

            def hid_mms(j, stop_last=False):
                ps, o = pscol(j)
                for kk in range(8):
                    mm(ps, o, 128, curT[j - 1][:, kk * B:(kk + 1) * B],
                       hid_sb[j][:, kk * 128:(kk + 1) * 128],
                       stop=stop_last and kk == 7)

            def skip_mms(c, T):
                st = skip_sb[(c, T)]
                wid = 128 * len(T)
                ps, o = pscol(T[0])
                for kk in range(8):
                    mm(ps, o, wid, curT[c][:, kk * B:(kk + 1) * B],
                       st[:, kk * wid:(kk + 1) * wid])

            def chain_tail(j):
                """sigmoid -> AllGather (untransposed) -> X-bar transpose.

                The [64, 128] sigmoid output goes straight to the collective;
                the gathered [512, 128] comes back through one HWDGE
                transpose-DMA that lands directly in curT layout.  This
                removes the PE transpose, DVE copy, and rearrange unpack
                from the serial chain."""
                ps, o = pscol(j)
                cp = curp.tile([64, 128], bf16, name=f"cp{j}", tag=f"cp{j}")
                nc.scalar.activation(cp, ps[:, o:o + 128], AF.Sigmoid)
                agin = dramp.tile([64, 128], bf16, name=f"agin{j}", tag=f"agin{j}")
                agout = dramp.tile([8 * 64, 128], bf16, name=f"agout{j}",
                                   tag=f"agout{j}", addr_space="Shared")
                nc.scalar.dma_start(out=agin, in_=cp)
                nc.gpsimd.collective_compute(
                    "AllGather",
                    mybir.AluOpType.bypass,
                    replica_groups=rg,
                    ins=[agin.opt()],
                    outs=[agout.opt()],
                )
                dst = curp.tile([128, 8 * B], bf16, name=f"curT{j}", tag=f"curT{j}")
                nc.scalar.dma_start_transpose(dst[:, :], agout[:, :])
                curT[j] = dst

            def await_round(j):
                pass

            # ---- PE emission order (chain + window back-fill) ----------
            rec_mms(rec0_sb, 0, 128)
            chain_tail(0)

            rec_mms(rec1_sb, 128, 128)         # AG_0 window
            await_round(0)
            hid_mms(1)                          # urgent (needs AG_0)
            chain_tail(1)

            rec_mms(rec23_sb, 256, 256)        # AG_1 window
            skip_mms(0, (2, 3))
            await_round(1)
            skip_mms(1, (2,))                   # urgent (needs AG_1)
            hid_mms(2)
            chain_tail(2)

            rec_mms(rec456_sb, 512, 384)       # AG_2 window
            skip_mms(1, (3,))
            skip_mms(0, (4, 5, 6, 7))
            skip_mms(1, (4, 5, 6, 7))
            await_round(2)
            skip_mms(2, (3,))                   # urgent (needs AG_2)
            hid_mms(3, stop_last=True)          # closes psA
            chain_tail(3)

            skip_mms(2, (4, 5, 6, 7))          # AG_3 window
            await_round(3)
            skip_mms(3, (4,))                   # urgent (needs AG_3)
            hid_mms(4)
            chain_tail(4)

            skip_mms(3, (5, 6, 7))             # AG_4 window
            await_round(4)
            skip_mms(4, (5,))                   # urgent
            hid_mms(5)
            chain_tail(5)

            skip_mms(4, (6, 7))                # AG_5 window
            await_round(5)
            skip_mms(5, (6,))                   # urgent
            hid_mms(6)
            chain_tail(6)

            skip_mms(5, (7,))                  # AG_6 window
            await_round(6)
            skip_mms(6, (7,))                   # urgent
            hid_mms(7, stop_last=True)          # closes psB

            # ---- block 7 tail: AllGather cur7 like every other block and
            # compute the full [64, 512] output redundantly on each core.
            # The AG fires right after sigmoid_7 (no psum serialization) and
            # an AG of 16KB beats a ReduceScatter of 128KB by ~5us.
            chain_tail(7)
            pso = psump.tile([64, 512], fp32, name="pso", tag="pso")
            nc.tensor.matmul(
                pso, lhsT=ones_sb[:, :], rhs=boutrow_sb[:, :], start=True,
                stop=False)
            for kk in range(8):
                nc.tensor.matmul(
                    pso, lhsT=curT[7][:, kk * B:(kk + 1) * B],
                    rhs=wout_sb[:, kk * 512:(kk + 1) * 512],
                    start=False, stop=(kk == 7))
            out_sb = curp.tile([64, 512], fp32, name="out_sb", tag="out_sb")
            nc.vector.tensor_copy(out_sb, pso)
            nc.scalar.dma_start(out=t_out[:, :], in_=out_sb)

    nc.compile()
    return nc


def _get_nc():
    global _BUILT
    if _BUILT is None:
        _BUILT = _build()
    return _BUILT


def make_in_maps(x, hidden_states, W_in, b_in, W_hid, b_hid, W_rec, W_skip,
                 W_out, b_out, mask_hid, mask_rec, mask_skip):
    x = np.asarray(x, np.float32)
    h = np.asarray(hidden_states, np.float32)
    W_in = np.asarray(W_in, np.float32)
    b_in = np.asarray(b_in, np.float32)
    W_out = np.asarray(W_out, np.float32)
    b_out = np.asarray(b_out, np.float32)
    Wh = np.asarray(W_hid, np.float32) * np.asarray(mask_hid, np.float32)
    Wr = np.asarray(W_rec, np.float32) * np.asarray(mask_rec, np.float32)
    Ws = np.asarray(W_skip, np.float32) * np.asarray(mask_skip, np.float32)
    b_hid = np.asarray(b_hid, np.float32)

    hT = _pack(h, FP8)
    xT = _pack(x, BF16)
    ones = np.ones((1, B), BF16)
    ident = np.eye(B).astype(BF16)
    boutrow = np.ascontiguousarray(b_out[None, :]).astype(BF16)

    in_maps = []
    for c_ in range(NCORES):
        R = slice(c_ * RP, (c_ + 1) * RP)
        biases = [b_in[R]] + [b_hid[i, R] for i in range(7)]
        biasrow = np.zeros((1, 8 * 128), np.float32)
        for j in range(8):
            biasrow[0, j * 128:(j + 1) * 128] = biases[j]
        m = {
            "hT": hT,
            "xT": xT,
            "win": _pack(W_in[R], BF16),
            "rec0": _pack(Wr[0, R], FP8),
            "rec1": _pack(Wr[1, R], FP8),
            "rec23": _pack_group([Wr[2, R], Wr[3, R]], FP8),
            "rec456": _pack_group([Wr[4, R], Wr[5, R], Wr[6, R]], FP8),
            "biasrow": biasrow.astype(BF16),
            "wout": _pack(W_out, BF16),
            "boutrow": boutrow,
            "ones": ones,
            "ident": ident,
        }
        for j in range(1, 8):
            m[f"hid{j}"] = _pack(Wh[j - 1, R], BF16)
        for c, T in SKIP_GROUPS:
            mats = [Ws[t - 2, R, c * W:(c + 1) * W] for t in T]
            m[f"skip{c}_{''.join(map(str, T))}"] = _pack_group(mats, FP8)
        in_maps.append(m)
    return in_maps


def run(in_maps, **kw):
    from concourse import bass_utils
    nc = _get_nc()
    return bass_utils.run_bass_kernel_spmd(
        nc, in_maps, core_ids=list(range(NCORES)), **kw
    )


def kernel(**inputs):
    in_maps = make_in_maps(**inputs)
    res = run(in_maps)
    return np.ascontiguousarray(res.results[0]["out"], dtype=np.float32)
